# revision 39
# baseline (speedup 1.0000x reference)
"""Trainium2 Bass kernel for causal retention multi-head attention.

    kqv = x @ W1 + b1 ; k,q,v = split(kqv)
    out_h = tril_mask(q_h k_h^T) v_h        (per head, no softmax)
    out = concat_heads @ W2 + b2

No softmax means the attention is LINEAR: out[t] = q_t @ sum_{s<=t} k_s v_s^T.
Cross-chunk context is carried by a per-head 64x64 running state
S = sum K_j^T V_j accumulated in PSUM (both heads packed block-diagonally in
one 128x128 tile); only the in-chunk 4x(128-key) diagonal blocks need
explicit masked scores. This removes the O(T^2) score matmuls and PSUM
evacuations of the direct approach.

Sharding: tensor-parallel over heads. Each of the 8 cores computes 2 heads
(128 feature columns of the attention output), then an AllToAll exchanges
head-feature slices for sequence slices so each core applies the full W2 to
its 512 rows of the sequence.

All matmuls run in bf16 with fp32 PSUM accumulation.
"""
import numpy as np

import concourse.bacc as bacc
import concourse.bass as bass
import concourse.mybir as mybir
import concourse.tile as tile
from concourse.bass_utils import run_bass_kernel_spmd

dt = mybir.dt
ts = bass.ts

T = 4096          # sequence length
D = 1024          # embed dim
NCORES = 8
FPC = D // NCORES  # feature (head-dim) columns per core = 128 (2 heads x 64)
TC = 512          # t-chunk (query block)
NT = T // TC      # 8 t-chunks
SB = 128          # s-block (key block)

# A2A exchange groups: (chunk list, shard width, out row base)
GROUPS_SPEC = {
    "asc": [([0, 1, 2, 3], 256, 0), ([4, 5, 6, 7], 256, 256)],
}


class EngineBalancer:
    """Greedy assignment of PSUM->SBUF copies to the less-loaded of DVE/ACT."""

    def __init__(self, nc):
        self.nc = nc
        self.busy = {"dve": 0.0, "act": 1.3}
        self.act_unit = 0.555  # ACT pays a one-time LUT load

    def copy(self, out_ap, in_ap, cols):
        dve_cost = 0.569 * cols / 512
        act_cost = self.act_unit * cols / 512
        if self.busy["dve"] + dve_cost <= self.busy["act"] + act_cost:
            self.busy["dve"] += dve_cost
            self.nc.vector.tensor_copy(out_ap, in_ap)
        else:
            self.busy["act"] += act_cost
            self.nc.scalar.copy(out_ap, in_ap)

    def mul_mask(self, out_ap, in_ap, mask_ap, cols):
        # tensor_tensor only exists on DVE (ACT has no two-tensor op)
        self.busy["dve"] += 0.569 * cols / 512
        self.nc.vector.tensor_mul(out_ap, in_ap, mask_ap)

    def bias_add(self, out_ap, in_ap, bias_ap, cols):
        # per-partition bias: ACT activation(Identity) or DVE tensor_scalar_add
        dve_cost = 0.569 * cols / 512
        act_cost = self.act_unit * cols / 512
        if self.busy["dve"] + dve_cost <= self.busy["act"] + act_cost:
            self.busy["dve"] += dve_cost
            self.nc.vector.tensor_scalar_add(out_ap, in_ap, bias_ap)
        else:
            self.busy["act"] += act_cost
            self.nc.scalar.activation(
                out_ap, in_ap, mybir.ActivationFunctionType.Identity,
                bias=bias_ap, scale=1.0)

    def tensor_add(self, out_ap, a_ap, b_ap, cols):
        self.busy["dve"] += 0.569 * cols / 512
        self.nc.vector.tensor_add(out_ap, a_ap, b_ap)


def build_program(reps: int = 1, act_unit: float = 0.612,
                  groups_mode: str = 'asc', spbufs: int = 2,
                  knt_dma: bool = True, fake_cc: bool = False) -> bass.Bass:
    nc = bacc.Bacc("TRN2", target_bir_lowering=False, debug=False,
                   num_devices=NCORES)

    xT = nc.dram_tensor("xT", [D, T], dt.bfloat16, kind="ExternalInput")
    w1l = nc.dram_tensor("w1l", [D, 3 * FPC], dt.bfloat16, kind="ExternalInput")
    b1l = nc.dram_tensor("b1l", [3, FPC], dt.float32, kind="ExternalInput")
    w2 = nc.dram_tensor("w2", [D, D], dt.bfloat16, kind="ExternalInput")
    b2 = nc.dram_tensor("b2", [D], dt.float32, kind="ExternalInput")
    masks = nc.dram_tensor("masks", [4, SB, TC], dt.float32, kind="ExternalInput")
    ident = nc.dram_tensor("ident", [SB, SB], dt.bfloat16, kind="ExternalInput")
    out = nc.dram_tensor("out", [TC, D], dt.bfloat16, kind="ExternalOutput")

    rg = [list(range(NCORES))]

    with tile.TileContext(nc) as tc:
        with (
            tc.tile_pool(name="res", bufs=1) as res,          # resident tensors
            tc.tile_pool(name="sp0", bufs=spbufs) as sp0,     # S tiles head A
            tc.tile_pool(name="sp1", bufs=spbufs) as sp1,     # S tiles head B
            tc.tile_pool(name="stp", bufs=2) as stp,          # state snapshots
            tc.tile_pool(name="stage", bufs=3) as stage,
            tc.tile_pool(name="ost", bufs=4) as ostp,         # output staging
            tc.tile_pool(name="rx", bufs=16) as rxp,          # post-A2A tiles
            tc.tile_pool(name="psA", bufs=2, space="PSUM") as psA,
            tc.tile_pool(name="psS0", bufs=2, space="PSUM") as psS0,
            tc.tile_pool(name="psS1", bufs=2, space="PSUM") as psS1,
            tc.tile_pool(name="psO", bufs=1, space="PSUM") as psO,
            tc.tile_pool(name="psSt", bufs=1, space="PSUM") as psStp,
            tc.tile_pool(name="dram", bufs=1, space="DRAM") as dram,
        ):
            eb = EngineBalancer(nc)
            eb.act_unit = act_unit
            # ---- resident loads ------------------------------------------------
            w1sb = res.tile([128, D // 128, 3 * FPC], dt.bfloat16, tag="w1")
            nc.sync.dma_start(w1sb[:], w1l.ap().rearrange("(a p) c -> p a c", p=128))
            b1sb = res.tile([FPC, 3], dt.float32, tag="b1")
            nc.sync.dma_start(b1sb[:], b1l.ap().rearrange("m p -> p m"))
            identsb = res.tile([SB, SB], dt.bfloat16, tag="ident")
            nc.sync.dma_start(identsb[:], ident.ap())
            xsb = res.tile([128, D // 128, T], dt.bfloat16, tag="x", name="xsb")
            for kc in range(D // 128):
                nc.sync.dma_start(xsb[:, kc, :],
                                  xT[kc * 128:(kc + 1) * 128, :])
            masksb = res.tile([SB, 4, TC], dt.float32, tag="masks")
            nc.sync.dma_start(masksb[:], masks.ap().rearrange("r p c -> p r c"))
            w2sb = res.tile([128, D // 128, D], dt.bfloat16, tag="w2")
            b2row = res.tile([1, D], dt.float32, tag="b2row")
            b2bc = res.tile([128, D], dt.float32, tag="b2bc")
            nc.sync.dma_start(w2sb[:], w2.ap().rearrange("(a p) c -> p a c", p=128))
            nc.sync.dma_start(b2row[:], b2.ap().rearrange("(o c) -> o c", o=1))
            nc.gpsimd.partition_broadcast(b2bc[:], b2row[:])

            kTsb = res.tile([FPC, T], dt.bfloat16, tag="kT")
            qTsb = res.tile([FPC, T], dt.bfloat16, tag="qT")
            vsb = res.tile([SB, T // SB, FPC], dt.bfloat16, tag="v")
            # k in [t, d] layout (for the state update); last chunk not needed
            knT = res.tile([SB, (NT - 1) * 4, FPC], dt.bfloat16, tag="knT")
            # running state: both heads packed block-diagonally
            psSt = psStp.tile([128, 128], dt.float32, tag="st", name="psSt")

            # A2A exchange groups: (chunk list, shard width, out row base).
            groups = GROUPS_SPEC[groups_mode]
            a2a_in, a2a_out = [], []
            for g, (chunks, W, base) in enumerate(groups):
                a2a_in.append(dram.tile([NCORES, FPC, W], dt.bfloat16,
                                        tag=f"ain{g}", name=f"a2a_in{g}"))
                a2a_out.append(dram.tile([NCORES, FPC, W], dt.bfloat16,
                                         tag=f"aout{g}", name=f"a2a_out{g}"))
            chunk_group = {}
            for g, (chunks, W, base) in enumerate(groups):
                for pos, i in enumerate(chunks):
                    chunk_group[i] = (g, pos)

            def a2a_launch(g: int):
                if fake_cc:
                    nc.gpsimd.dma_start(a2a_out[g][:], a2a_in[g][:])
                    return
                nc.gpsimd.collective_compute(
                    "AllToAll", mybir.AluOpType.bypass, replica_groups=rg,
                    ins=[a2a_in[g].opt()], outs=[a2a_out[g].opt()],
                )

            def rx_load(g: int):
                W = groups[g][1]
                rxt = []
                for fc in range(NCORES):
                    r = rxp.tile([FPC, W], dt.bfloat16, tag="rx",
                                 name=f"rx{g}_{fc}")
                    nc.sync.dma_start(r[:], a2a_out[g][fc])
                    rxt.append(r)
                return rxt

            def att_store(i, att):
                g, pos = chunk_group[i]
                W = groups[g][1]
                a = 512 * pos
                while a < 512 * pos + TC:
                    s = a // W
                    e = min((s + 1) * W, 512 * pos + TC)
                    nc.sync.dma_start(
                        a2a_in[g][s][:, a - s * W: e - s * W],
                        att[:, a - 512 * pos: e - 512 * pos])
                    a = e

            def w2_group(g: int, rxt):
                """Project group g's rows through W2 (after its AllToAll)."""
                chunks, W, base = groups[g]
                for tt in range(W // 128):
                    for ec in range(2):
                        psF = psA.tile([128, 512], dt.float32, tag="acc",
                                       name=f"psF_{g}_{tt}_{ec}")
                        for fc in range(NCORES):
                            nc.tensor.matmul(
                                psF[:], rxt[fc][:, ts(tt, 128)],
                                w2sb[:, fc, ts(ec, 512)],
                                start=(fc == 0), stop=(fc == NCORES - 1),
                            )
                        o = ostp.tile([128, 512], dt.bfloat16, tag="ost",
                                      name=f"o_{g}_{tt}_{ec}")
                        eb.tensor_add(o[:], psF[:], b2bc[:, ts(ec, 512)], 512)
                        nc.sync.dma_start(
                            out[base + tt * 128: base + tt * 128 + 128,
                                ts(ec, 512)], o[:])

            def phase_a(i: int):
                """kqv^T for t-chunk i."""
                xts = [xsb[:, kc, ts(i, TC)] for kc in range(D // 128)]
                for m in range(3):
                    ps = psA.tile([128, TC], dt.float32, tag="acc",
                                  name=f"psa_{i}_{m}")
                    for kc in range(D // 128):
                        nc.tensor.matmul(
                            ps[:], w1sb[:, kc, ts(m, FPC)], xts[kc],
                            start=(kc == 0), stop=(kc == D // 128 - 1),
                        )
                    if m == 0:
                        eb.bias_add(kTsb[:, ts(i, TC)], ps[:],
                                    b1sb[:, m:m + 1], TC)
                        if i < NT - 1:
                            # k in [t, d] layout via the DMA xbar transpose
                            for tb in range(4):
                                nc.sync.dma_start_transpose(
                                    knT[:, 4 * i + tb, :],
                                    kTsb[:, i * TC + tb * SB:
                                         i * TC + (tb + 1) * SB])
                    elif m == 1:
                        eb.bias_add(qTsb[:, ts(i, TC)], ps[:],
                                    b1sb[:, m:m + 1], TC)
                    else:
                        vtmp = stage.tile([128, TC], dt.bfloat16, tag="vtmp",
                                          name=f"vtmp_{i}")
                        eb.bias_add(vtmp[:], ps[:], b1sb[:, m:m + 1], TC)
                        for u in range(TC // SB):
                            pst = psA.tile([SB, SB], dt.bfloat16, tag="acc",
                                           name=f"pst_{i}_{u}")
                            nc.tensor.transpose(
                                pst[:], vtmp[:, ts(u, SB)], identsb[:])
                            eb.copy(vsb[:, 4 * i + u, :], pst[:], 128)

            def attention(i: int):
                otp = psO.tile([128, TC], dt.float32, tag="ot",
                               name=f"otp_{i}")
                # ---- cross-chunk context via the running state -------------
                if i > 0:
                    st = stp.tile([128, 128], dt.bfloat16, tag="st",
                                  name=f"st_{i}")
                    eb.copy(st[:], psSt[:], 128)
                    nc.tensor.matmul(
                        otp[0:64, :], st[0:64, 0:64],
                        qTsb[0:64, ts(i, TC)], start=True, stop=True)
                    nc.tensor.matmul(
                        otp[64:128, :], st[64:128, 64:128],
                        qTsb[64:128, ts(i, TC)], start=True, stop=True)
                # ---- state update with this chunk's keys/values ------------
                # issued right after the apply so the cross-chunk serial
                # chain (evac -> apply/update -> next evac) isn't queued
                # behind the in-chunk score matmuls
                if i < NT - 1:
                    for tb in range(4):
                        nc.tensor.matmul(
                            psSt[:], knT[:, 4 * i + tb, :],
                            vsb[:, 4 * i + tb, :],
                            start=(i == 0 and tb == 0),
                            stop=(i == NT - 2 and tb == 3))
                # ---- in-chunk diagonal blocks ------------------------------
                for r in range(4):
                    j = 4 * i + r
                    cs = 128 * r
                    s0p = psS0.tile([128, TC], dt.float32, tag="s0",
                                    name=f"s0p_{i}_{r}")
                    s1p = psS1.tile([128, TC], dt.float32, tag="s1",
                                    name=f"s1p_{i}_{r}")
                    nc.tensor.matmul(
                        s0p[:, cs:TC], kTsb[0:64, ts(j, SB)],
                        qTsb[0:64, i * TC + cs:(i + 1) * TC],
                        start=True, stop=True, tile_position=(0, 0))
                    nc.tensor.matmul(
                        s1p[:, cs:TC], kTsb[64:128, ts(j, SB)],
                        qTsb[64:128, i * TC + cs:(i + 1) * TC],
                        start=True, stop=True, tile_position=(64, 0))
                    s0s = sp0.tile([128, TC], dt.bfloat16, tag="s0s",
                                   name=f"s0s_{i}_{r}")
                    s1s = sp1.tile([128, TC], dt.bfloat16, tag="s1s",
                                   name=f"s1s_{i}_{r}")
                    # the triangle lives in cols [128r, 128r+128); the rest of
                    # the row is a plain (engine-balanced) copy.
                    me = min(cs + 128, TC)
                    eb.mul_mask(s0s[:, cs:me], s0p[:, cs:me],
                                masksb[:, r, cs:me], me - cs)
                    eb.mul_mask(s1s[:, cs:me], s1p[:, cs:me],
                                masksb[:, r, cs:me], me - cs)
                    if me < TC:
                        eb.copy(s0s[:, me:TC], s0p[:, me:TC], TC - me)
                        eb.copy(s1s[:, me:TC], s1p[:, me:TC], TC - me)
                    nc.tensor.matmul(
                        otp[0:64, cs:TC], vsb[:, j, 0:64], s0s[:, cs:TC],
                        start=(i == 0 and r == 0), stop=(r == 3),
                        tile_position=(0, 0))
                    nc.tensor.matmul(
                        otp[64:128, cs:TC], vsb[:, j, 64:128], s1s[:, cs:TC],
                        start=(i == 0 and r == 0), stop=(r == 3),
                        tile_position=(0, 64))
                att = stage.tile([128, TC], dt.bfloat16, tag="att",
                                 name=f"att_{i}")
                # split halves across DVE/ACT so the psO bank frees sooner
                eb.copy(att[:, 0:TC // 2], otp[:, 0:TC // 2], TC // 2)
                eb.copy(att[:, TC // 2:TC], otp[:, TC // 2:TC], TC // 2)
                att_store(i, att)

            # ---- schedule ------------------------------------------------------
            for _rep in range(reps):
                phase_a(0)
                phase_a(1)
                rxt0 = None
                for i in range(NT):
                    attention(i)
                    if i + 2 < NT:
                        phase_a(i + 2)
                    if i == 3:
                        a2a_launch(0)
                    if i == 4:
                        rxt0 = rx_load(0)
                    if i == 5:
                        w2_group(0, rxt0)
                a2a_launch(1)
                rxt1 = rx_load(1)
                w2_group(1, rxt1)

    nc.compile()
    return nc


_GROUPS_MODE = "asc"
_PROGRAM_CACHE: list = []


def _get_program() -> bass.Bass:
    if not _PROGRAM_CACHE:
        _PROGRAM_CACHE.append(build_program(groups_mode=_GROUPS_MODE))
    return _PROGRAM_CACHE[0]


def _make_in_maps(x, W1, b1, W2, b2):
    import ml_dtypes
    bf16 = ml_dtypes.bfloat16
    xTb = np.ascontiguousarray(np.asarray(x, np.float32).T).astype(bf16)
    w2b = np.asarray(W2, np.float32).astype(bf16)
    b2f = np.ascontiguousarray(b2, dtype=np.float32)
    maskt = np.zeros((4, SB, TC), dtype=np.float32)
    for r in range(4):
        for p in range(SB):
            maskt[r, p, 128 * r + p:] = 1.0
    identm = np.eye(SB, dtype=np.float32).astype(bf16)

    in_maps = []
    for c in range(NCORES):
        cols = slice(FPC * c, FPC * (c + 1))
        w1c = np.concatenate(
            [W1[:, 0 * D:][:, cols], W1[:, 1 * D:][:, cols], W1[:, 2 * D:][:, cols]],
            axis=1)
        b1c = np.stack(
            [b1[0 * D:][cols], b1[1 * D:][cols], b1[2 * D:][cols]], axis=0)
        in_maps.append({
            "xT": xTb,
            "w1l": np.asarray(w1c, np.float32).astype(bf16),
            "b1l": np.ascontiguousarray(b1c, dtype=np.float32),
            "w2": w2b,
            "b2": b2f,
            "masks": maskt,
            "ident": identm,
        })
    return in_maps


def kernel(x, W1, b1, W2, b2, _trace=False, **trace_kwargs):
    x = np.asarray(x)
    W1, b1, W2, b2 = (np.asarray(a) for a in (W1, b1, W2, b2))
    nc = _get_program()
    in_maps = _make_in_maps(x, W1, b1, W2, b2)
    res = run_bass_kernel_spmd(
        nc, in_maps, core_ids=list(range(NCORES)), trace=_trace, **trace_kwargs)
    full = np.empty((T, D), dtype=np.float32)
    groups = GROUPS_SPEC[_GROUPS_MODE]
    for c in range(NCORES):
        o = np.asarray(res.results[c]["out"], dtype=np.float32)
        for chunks, W, base in groups:
            L = np.arange(W * c, W * c + W)
            tglob = 512 * np.asarray(chunks)[L // 512] + (L % 512)
            full[tglob] = o[base: base + W]
    if _trace:
        kernel.last_results = res
    return full


# revision 40
# speedup vs baseline: 1.2386x; 1.2386x over previous
"""Trainium2 Bass kernel for causal retention multi-head attention.

    kqv = x @ W1 + b1 ; k,q,v = split(kqv)
    out_h = tril_mask(q_h k_h^T) v_h        (per head, no softmax)
    out = concat_heads @ W2 + b2

No softmax means the attention is LINEAR: out[t] = q_t @ sum_{s<=t} k_s v_s^T.
Cross-chunk context is carried by a per-head 64x64 running state
S = sum K_j^T V_j accumulated in PSUM (both heads packed block-diagonally in
one 128x128 tile); only the in-chunk 4x(128-key) diagonal blocks need
explicit masked scores. This removes the O(T^2) score matmuls and PSUM
evacuations of the direct approach.

Sharding: tensor-parallel over heads. Each of the 8 cores computes 2 heads
(128 feature columns of the attention output), then an AllToAll exchanges
head-feature slices for sequence slices so each core applies the full W2 to
its 512 rows of the sequence.

All matmuls run in bf16 with fp32 PSUM accumulation.
"""
import numpy as np

import concourse.bacc as bacc
import concourse.bass as bass
import concourse.mybir as mybir
import concourse.tile as tile
from concourse.bass_utils import run_bass_kernel_spmd

dt = mybir.dt
ts = bass.ts

T = 4096          # sequence length
D = 1024          # embed dim
NCORES = 8
FPC = D // NCORES  # feature (head-dim) columns per core = 128 (2 heads x 64)
TC = 512          # t-chunk (query block)
NT = T // TC      # 8 t-chunks
SB = 128          # s-block (key block)

# A2A exchange groups: (chunk list, shard width, out row base)
GROUPS_SPEC = {
    "asc": [([0, 1, 2, 3], 256, 0), ([4, 5, 6, 7], 256, 256)],
}


class EngineBalancer:
    """Greedy assignment of PSUM->SBUF copies to the less-loaded of DVE/ACT."""

    def __init__(self, nc):
        self.nc = nc
        self.busy = {"dve": 0.0, "act": 1.3}
        self.act_unit = 0.555  # ACT pays a one-time LUT load

    def copy(self, out_ap, in_ap, cols):
        dve_cost = 0.569 * cols / 512
        act_cost = self.act_unit * cols / 512
        if self.busy["dve"] + dve_cost <= self.busy["act"] + act_cost:
            self.busy["dve"] += dve_cost
            self.nc.vector.tensor_copy(out_ap, in_ap)
        else:
            self.busy["act"] += act_cost
            self.nc.scalar.copy(out_ap, in_ap)

    def mul_mask(self, out_ap, in_ap, mask_ap, cols):
        # tensor_tensor only exists on DVE (ACT has no two-tensor op)
        self.busy["dve"] += 0.569 * cols / 512
        self.nc.vector.tensor_mul(out_ap, in_ap, mask_ap)

    def bias_add(self, out_ap, in_ap, bias_ap, cols):
        # per-partition bias: ACT activation(Identity) or DVE tensor_scalar_add
        dve_cost = 0.569 * cols / 512
        act_cost = self.act_unit * cols / 512
        if self.busy["dve"] + dve_cost <= self.busy["act"] + act_cost:
            self.busy["dve"] += dve_cost
            self.nc.vector.tensor_scalar_add(out_ap, in_ap, bias_ap)
        else:
            self.busy["act"] += act_cost
            self.nc.scalar.activation(
                out_ap, in_ap, mybir.ActivationFunctionType.Identity,
                bias=bias_ap, scale=1.0)

    def tensor_add(self, out_ap, a_ap, b_ap, cols):
        self.busy["dve"] += 0.569 * cols / 512
        self.nc.vector.tensor_add(out_ap, a_ap, b_ap)


def build_program(reps: int = 1, act_unit: float = 0.612,
                  groups_mode: str = 'asc', spbufs: int = 2,
                  knt_dma: bool = True, fake_cc: bool = False) -> bass.Bass:
    nc = bacc.Bacc("TRN2", target_bir_lowering=False, debug=False,
                   num_devices=NCORES)

    xT = nc.dram_tensor("xT", [D, T], dt.bfloat16, kind="ExternalInput")
    w1l = nc.dram_tensor("w1l", [D, 3 * FPC], dt.bfloat16, kind="ExternalInput")
    b1l = nc.dram_tensor("b1l", [3, FPC], dt.float32, kind="ExternalInput")
    w2 = nc.dram_tensor("w2", [D, D], dt.bfloat16, kind="ExternalInput")
    b2 = nc.dram_tensor("b2", [D], dt.float32, kind="ExternalInput")
    masks = nc.dram_tensor("masks", [4, SB, TC], dt.float32, kind="ExternalInput")
    ident = nc.dram_tensor("ident", [SB, SB], dt.bfloat16, kind="ExternalInput")
    out = nc.dram_tensor("out", [TC, D], dt.bfloat16, kind="ExternalOutput")

    rg = [list(range(NCORES))]

    with tile.TileContext(nc) as tc:
        with (
            tc.tile_pool(name="res", bufs=1) as res,          # resident tensors
            tc.tile_pool(name="sp0", bufs=spbufs) as sp0,     # S tiles head A
            tc.tile_pool(name="sp1", bufs=spbufs) as sp1,     # S tiles head B
            tc.tile_pool(name="stp", bufs=2) as stp,          # state snapshots
            tc.tile_pool(name="stage", bufs=3) as stage,
            tc.tile_pool(name="ost", bufs=4) as ostp,         # output staging
            tc.tile_pool(name="rx", bufs=16) as rxp,          # post-A2A tiles
            tc.tile_pool(name="psA", bufs=2, space="PSUM") as psA,
            tc.tile_pool(name="psS0", bufs=2, space="PSUM") as psS0,
            tc.tile_pool(name="psS1", bufs=2, space="PSUM") as psS1,
            tc.tile_pool(name="psO", bufs=1, space="PSUM") as psO,
            tc.tile_pool(name="psSt", bufs=1, space="PSUM") as psStp,
            tc.tile_pool(name="dram", bufs=1, space="DRAM") as dram,
        ):
            eb = EngineBalancer(nc)
            eb.act_unit = act_unit
            # ---- resident loads ------------------------------------------------
            w1sb = res.tile([128, D // 128, 3 * FPC], dt.bfloat16, tag="w1")
            nc.sync.dma_start(w1sb[:], w1l.ap().rearrange("(a p) c -> p a c", p=128))
            b1sb = res.tile([FPC, 3], dt.float32, tag="b1")
            nc.sync.dma_start(b1sb[:], b1l.ap().rearrange("m p -> p m"))
            identsb = res.tile([SB, SB], dt.bfloat16, tag="ident")
            nc.sync.dma_start(identsb[:], ident.ap())
            xsb = res.tile([128, D // 128, T], dt.bfloat16, tag="x", name="xsb")
            for kc in range(D // 128):
                nc.sync.dma_start(xsb[:, kc, :],
                                  xT[kc * 128:(kc + 1) * 128, :])
            masksb = res.tile([SB, 4, TC], dt.float32, tag="masks")
            nc.sync.dma_start(masksb[:], masks.ap().rearrange("r p c -> p r c"))
            w2sb = res.tile([128, D // 128, D], dt.bfloat16, tag="w2")
            b2row = res.tile([1, D], dt.float32, tag="b2row")
            b2bc = res.tile([128, D], dt.float32, tag="b2bc")
            nc.sync.dma_start(w2sb[:], w2.ap().rearrange("(a p) c -> p a c", p=128))
            nc.sync.dma_start(b2row[:], b2.ap().rearrange("(o c) -> o c", o=1))
            nc.gpsimd.partition_broadcast(b2bc[:], b2row[:])

            kTsb = res.tile([FPC, T], dt.bfloat16, tag="kT")
            qTsb = res.tile([FPC, T], dt.bfloat16, tag="qT")
            vsb = res.tile([SB, T // SB, FPC], dt.bfloat16, tag="v")
            # k in [t, d] layout (for the state update); last chunk not needed
            knT = res.tile([SB, (NT - 1) * 4, FPC], dt.bfloat16, tag="knT")
            # running state: both heads packed block-diagonally
            psSt = psStp.tile([128, 128], dt.float32, tag="st", name="psSt")

            # A2A exchange groups: (chunk list, shard width, out row base).
            groups = GROUPS_SPEC[groups_mode]
            a2a_in, a2a_out = [], []
            for g, (chunks, W, base) in enumerate(groups):
                a2a_in.append(dram.tile([NCORES, FPC, W], dt.bfloat16,
                                        tag=f"ain{g}", name=f"a2a_in{g}"))
                a2a_out.append(dram.tile([NCORES, FPC, W], dt.bfloat16,
                                         tag=f"aout{g}", name=f"a2a_out{g}"))
            chunk_group = {}
            for g, (chunks, W, base) in enumerate(groups):
                for pos, i in enumerate(chunks):
                    chunk_group[i] = (g, pos)

            def a2a_launch(g: int):
                if fake_cc:
                    nc.gpsimd.dma_start(a2a_out[g][:], a2a_in[g][:])
                    return
                nc.gpsimd.collective_compute(
                    "AllToAll", mybir.AluOpType.bypass, replica_groups=rg,
                    ins=[a2a_in[g].opt()], outs=[a2a_out[g].opt()],
                )

            def rx_load(g: int):
                W = groups[g][1]
                rxt = []
                for fc in range(NCORES):
                    r = rxp.tile([FPC, W], dt.bfloat16, tag="rx",
                                 name=f"rx{g}_{fc}")
                    nc.sync.dma_start(r[:], a2a_out[g][fc])
                    rxt.append(r)
                return rxt

            def att_store(i, att):
                g, pos = chunk_group[i]
                W = groups[g][1]
                a = 512 * pos
                while a < 512 * pos + TC:
                    s = a // W
                    e = min((s + 1) * W, 512 * pos + TC)
                    nc.sync.dma_start(
                        a2a_in[g][s][:, a - s * W: e - s * W],
                        att[:, a - 512 * pos: e - 512 * pos])
                    a = e

            def w2_group(g: int, rxt):
                """Project group g's rows through W2 (after its AllToAll)."""
                chunks, W, base = groups[g]
                for tt in range(W // 128):
                    for ec in range(2):
                        psF = psA.tile([128, 512], dt.float32, tag="acc",
                                       name=f"psF_{g}_{tt}_{ec}")
                        for fc in range(NCORES):
                            nc.tensor.matmul(
                                psF[:], rxt[fc][:, ts(tt, 128)],
                                w2sb[:, fc, ts(ec, 512)],
                                start=(fc == 0), stop=(fc == NCORES - 1),
                            )
                        o = ostp.tile([128, 512], dt.bfloat16, tag="ost",
                                      name=f"o_{g}_{tt}_{ec}")
                        eb.tensor_add(o[:], psF[:], b2bc[:, ts(ec, 512)], 512)
                        nc.sync.dma_start(
                            out[base + tt * 128: base + tt * 128 + 128,
                                ts(ec, 512)], o[:])

            def phase_a(i: int):
                """kqv^T for t-chunk i."""
                xts = [xsb[:, kc, ts(i, TC)] for kc in range(D // 128)]
                for m in range(3):
                    ps = psA.tile([128, TC], dt.float32, tag="acc",
                                  name=f"psa_{i}_{m}")
                    for kc in range(D // 128):
                        nc.tensor.matmul(
                            ps[:], w1sb[:, kc, ts(m, FPC)], xts[kc],
                            start=(kc == 0), stop=(kc == D // 128 - 1),
                        )
                    if m == 0:
                        eb.bias_add(kTsb[:, ts(i, TC)], ps[:],
                                    b1sb[:, m:m + 1], TC)
                        if i < NT - 1:
                            # k in [t, d] layout via the DMA xbar transpose
                            for tb in range(4):
                                nc.sync.dma_start_transpose(
                                    knT[:, 4 * i + tb, :],
                                    kTsb[:, i * TC + tb * SB:
                                         i * TC + (tb + 1) * SB])
                    elif m == 1:
                        eb.bias_add(qTsb[:, ts(i, TC)], ps[:],
                                    b1sb[:, m:m + 1], TC)
                    else:
                        vtmp = stage.tile([128, TC], dt.bfloat16, tag="vtmp",
                                          name=f"vtmp_{i}")
                        eb.bias_add(vtmp[:], ps[:], b1sb[:, m:m + 1], TC)
                        for u in range(TC // SB):
                            pst = psA.tile([SB, SB], dt.bfloat16, tag="acc",
                                           name=f"pst_{i}_{u}")
                            nc.tensor.transpose(
                                pst[:], vtmp[:, ts(u, SB)], identsb[:])
                            eb.copy(vsb[:, 4 * i + u, :], pst[:], 128)

            def attention(i: int):
                otp = psO.tile([128, TC], dt.float32, tag="ot",
                               name=f"otp_{i}")
                # ---- cross-chunk context via the running state -------------
                if i > 0:
                    st = stp.tile([128, 128], dt.bfloat16, tag="st",
                                  name=f"st_{i}")
                    eb.copy(st[:], psSt[:], 128)
                    nc.tensor.matmul(
                        otp[0:64, :], st[0:64, 0:64],
                        qTsb[0:64, ts(i, TC)], start=True, stop=True)
                    nc.tensor.matmul(
                        otp[64:128, :], st[64:128, 64:128],
                        qTsb[64:128, ts(i, TC)], start=True, stop=True)
                # ---- in-chunk diagonal blocks ------------------------------
                for r in range(4):
                    j = 4 * i + r
                    cs = 128 * r
                    s0p = psS0.tile([128, TC], dt.float32, tag="s0",
                                    name=f"s0p_{i}_{r}")
                    s1p = psS1.tile([128, TC], dt.float32, tag="s1",
                                    name=f"s1p_{i}_{r}")
                    nc.tensor.matmul(
                        s0p[:, cs:TC], kTsb[0:64, ts(j, SB)],
                        qTsb[0:64, i * TC + cs:(i + 1) * TC],
                        start=True, stop=True, tile_position=(0, 0))
                    nc.tensor.matmul(
                        s1p[:, cs:TC], kTsb[64:128, ts(j, SB)],
                        qTsb[64:128, i * TC + cs:(i + 1) * TC],
                        start=True, stop=True, tile_position=(64, 0))
                    s0s = sp0.tile([128, TC], dt.bfloat16, tag="s0s",
                                   name=f"s0s_{i}_{r}")
                    s1s = sp1.tile([128, TC], dt.bfloat16, tag="s1s",
                                   name=f"s1s_{i}_{r}")
                    # the triangle lives in cols [128r, 128r+128); the rest of
                    # the row is a plain (engine-balanced) copy.
                    me = min(cs + 128, TC)
                    eb.mul_mask(s0s[:, cs:me], s0p[:, cs:me],
                                masksb[:, r, cs:me], me - cs)
                    eb.mul_mask(s1s[:, cs:me], s1p[:, cs:me],
                                masksb[:, r, cs:me], me - cs)
                    if me < TC:
                        eb.copy(s0s[:, me:TC], s0p[:, me:TC], TC - me)
                        eb.copy(s1s[:, me:TC], s1p[:, me:TC], TC - me)
                    nc.tensor.matmul(
                        otp[0:64, cs:TC], vsb[:, j, 0:64], s0s[:, cs:TC],
                        start=(i == 0 and r == 0), stop=(r == 3),
                        tile_position=(0, 0))
                    nc.tensor.matmul(
                        otp[64:128, cs:TC], vsb[:, j, 64:128], s1s[:, cs:TC],
                        start=(i == 0 and r == 0), stop=(r == 3),
                        tile_position=(0, 64))
                # ---- state update with this chunk's keys/values ------------
                if i < NT - 1:
                    for tb in range(4):
                        nc.tensor.matmul(
                            psSt[:], knT[:, 4 * i + tb, :],
                            vsb[:, 4 * i + tb, :],
                            start=(i == 0 and tb == 0),
                            stop=(i == NT - 2 and tb == 3))

                att = stage.tile([128, TC], dt.bfloat16, tag="att",
                                 name=f"att_{i}")
                eb.copy(att[:], otp[:], TC)
                att_store(i, att)

            # ---- schedule ------------------------------------------------------
            for _rep in range(reps):
                phase_a(0)
                phase_a(1)
                rxt0 = None
                for i in range(NT):
                    attention(i)
                    if i + 2 < NT:
                        phase_a(i + 2)
                    if i == 3:
                        a2a_launch(0)
                    if i == 4:
                        rxt0 = rx_load(0)
                    if i == 5:
                        w2_group(0, rxt0)
                a2a_launch(1)
                rxt1 = rx_load(1)
                w2_group(1, rxt1)

    nc.compile()
    return nc


_GROUPS_MODE = "asc"
_PROGRAM_CACHE: list = []


def _get_program() -> bass.Bass:
    if not _PROGRAM_CACHE:
        _PROGRAM_CACHE.append(build_program(groups_mode=_GROUPS_MODE))
    return _PROGRAM_CACHE[0]


def _make_in_maps(x, W1, b1, W2, b2):
    import ml_dtypes
    bf16 = ml_dtypes.bfloat16
    xTb = np.ascontiguousarray(np.asarray(x, np.float32).T).astype(bf16)
    w2b = np.asarray(W2, np.float32).astype(bf16)
    b2f = np.ascontiguousarray(b2, dtype=np.float32)
    maskt = np.zeros((4, SB, TC), dtype=np.float32)
    for r in range(4):
        for p in range(SB):
            maskt[r, p, 128 * r + p:] = 1.0
    identm = np.eye(SB, dtype=np.float32).astype(bf16)

    in_maps = []
    for c in range(NCORES):
        cols = slice(FPC * c, FPC * (c + 1))
        w1c = np.concatenate(
            [W1[:, 0 * D:][:, cols], W1[:, 1 * D:][:, cols], W1[:, 2 * D:][:, cols]],
            axis=1)
        b1c = np.stack(
            [b1[0 * D:][cols], b1[1 * D:][cols], b1[2 * D:][cols]], axis=0)
        in_maps.append({
            "xT": xTb,
            "w1l": np.asarray(w1c, np.float32).astype(bf16),
            "b1l": np.ascontiguousarray(b1c, dtype=np.float32),
            "w2": w2b,
            "b2": b2f,
            "masks": maskt,
            "ident": identm,
        })
    return in_maps


def kernel(x, W1, b1, W2, b2, _trace=False, **trace_kwargs):
    x = np.asarray(x)
    W1, b1, W2, b2 = (np.asarray(a) for a in (W1, b1, W2, b2))
    nc = _get_program()
    in_maps = _make_in_maps(x, W1, b1, W2, b2)
    res = run_bass_kernel_spmd(
        nc, in_maps, core_ids=list(range(NCORES)), trace=_trace, **trace_kwargs)
    full = np.empty((T, D), dtype=np.float32)
    groups = GROUPS_SPEC[_GROUPS_MODE]
    for c in range(NCORES):
        o = np.asarray(res.results[c]["out"], dtype=np.float32)
        for chunks, W, base in groups:
            L = np.arange(W * c, W * c + W)
            tglob = 512 * np.asarray(chunks)[L // 512] + (L % 512)
            full[tglob] = o[base: base + W]
    if _trace:
        kernel.last_results = res
    return full


# revision 43
# speedup vs baseline: 1.9358x; 1.5629x over previous
"""Trainium2 Bass kernel for causal retention multi-head attention.

    kqv = x @ W1 + b1 ; k,q,v = split(kqv)
    out_h = tril_mask(q_h k_h^T) v_h        (per head, no softmax)
    out = concat_heads @ W2 + b2

No softmax means the attention is LINEAR: out[t] = q_t @ sum_{s<=t} k_s v_s^T.
Cross-chunk context is carried by a per-head 64x64 running state
S = sum K_j^T V_j accumulated in PSUM (both heads packed block-diagonally in
one 128x128 tile); only the in-chunk 4x(128-key) diagonal blocks need
explicit masked scores. This removes the O(T^2) score matmuls and PSUM
evacuations of the direct approach.

Sharding: tensor-parallel over heads. Each of the 8 cores computes 2 heads
(128 feature columns of the attention output), then an AllToAll exchanges
head-feature slices for sequence slices so each core applies the full W2 to
its 512 rows of the sequence.

All matmuls run in bf16 with fp32 PSUM accumulation.
"""
import numpy as np

import concourse.bacc as bacc
import concourse.bass as bass
import concourse.mybir as mybir
import concourse.tile as tile
from concourse.bass_utils import run_bass_kernel_spmd

dt = mybir.dt
ts = bass.ts

T = 4096          # sequence length
D = 1024          # embed dim
NCORES = 8
FPC = D // NCORES  # feature (head-dim) columns per core = 128 (2 heads x 64)
TC = 512          # t-chunk (query block)
NT = T // TC      # 8 t-chunks
SB = 128          # s-block (key block)

# A2A exchange groups: (chunk list, shard width, out row base)
GROUPS_SPEC = {
    "asc": [([0, 1, 2, 3], 256, 0), ([4, 5, 6, 7], 256, 256)],
}


class EngineBalancer:
    """Greedy assignment of PSUM->SBUF copies to the less-loaded of DVE/ACT."""

    def __init__(self, nc):
        self.nc = nc
        self.busy = {"dve": 0.0, "act": 1.3}
        self.act_unit = 0.555  # ACT pays a one-time LUT load

    def copy(self, out_ap, in_ap, cols):
        dve_cost = 0.569 * cols / 512
        act_cost = self.act_unit * cols / 512
        if self.busy["dve"] + dve_cost <= self.busy["act"] + act_cost:
            self.busy["dve"] += dve_cost
            self.nc.vector.tensor_copy(out_ap, in_ap)
        else:
            self.busy["act"] += act_cost
            self.nc.scalar.copy(out_ap, in_ap)

    def mul_mask(self, out_ap, in_ap, mask_ap, cols):
        # tensor_tensor only exists on DVE (ACT has no two-tensor op)
        self.busy["dve"] += 0.569 * cols / 512
        self.nc.vector.tensor_mul(out_ap, in_ap, mask_ap)

    def bias_add(self, out_ap, in_ap, bias_ap, cols):
        # per-partition bias: ACT activation(Identity) or DVE tensor_scalar_add
        dve_cost = 0.569 * cols / 512
        act_cost = self.act_unit * cols / 512
        if self.busy["dve"] + dve_cost <= self.busy["act"] + act_cost:
            self.busy["dve"] += dve_cost
            self.nc.vector.tensor_scalar_add(out_ap, in_ap, bias_ap)
        else:
            self.busy["act"] += act_cost
            self.nc.scalar.activation(
                out_ap, in_ap, mybir.ActivationFunctionType.Identity,
                bias=bias_ap, scale=1.0)

    def tensor_add(self, out_ap, a_ap, b_ap, cols):
        self.busy["dve"] += 0.569 * cols / 512
        self.nc.vector.tensor_add(out_ap, a_ap, b_ap)


def build_program(reps: int = 1, act_unit: float = 0.612,
                  groups_mode: str = 'asc', spbufs: int = 2,
                  knt_dma: bool = True, fake_cc: bool = False) -> bass.Bass:
    nc = bacc.Bacc("TRN2", target_bir_lowering=False, debug=False,
                   num_devices=NCORES)

    xT = nc.dram_tensor("xT", [D, T], dt.bfloat16, kind="ExternalInput")
    w1l = nc.dram_tensor("w1l", [D, 3 * FPC], dt.bfloat16, kind="ExternalInput")
    b1l = nc.dram_tensor("b1l", [3, FPC], dt.float32, kind="ExternalInput")
    w2 = nc.dram_tensor("w2", [D, D], dt.bfloat16, kind="ExternalInput")
    b2 = nc.dram_tensor("b2", [D], dt.float32, kind="ExternalInput")
    masks = nc.dram_tensor("masks", [4, SB, TC], dt.float32, kind="ExternalInput")
    ident = nc.dram_tensor("ident", [SB, SB], dt.bfloat16, kind="ExternalInput")
    out = nc.dram_tensor("out", [TC, D], dt.bfloat16, kind="ExternalOutput")

    rg = [list(range(NCORES))]

    with tile.TileContext(nc) as tc:
        with (
            tc.tile_pool(name="res", bufs=1) as res,          # resident tensors
            tc.tile_pool(name="sp0", bufs=spbufs) as sp0,     # S tiles head A
            tc.tile_pool(name="sp1", bufs=spbufs) as sp1,     # S tiles head B
            tc.tile_pool(name="stp", bufs=2) as stp,          # state snapshots
            tc.tile_pool(name="stage", bufs=3) as stage,
            tc.tile_pool(name="ost", bufs=4) as ostp,         # output staging
            tc.tile_pool(name="rx", bufs=16) as rxp,          # post-A2A tiles
            tc.tile_pool(name="psA", bufs=2, space="PSUM") as psA,
            tc.tile_pool(name="psS0", bufs=2, space="PSUM") as psS0,
            tc.tile_pool(name="psS1", bufs=2, space="PSUM") as psS1,
            tc.tile_pool(name="psO", bufs=1, space="PSUM") as psO,
            tc.tile_pool(name="psSt", bufs=1, space="PSUM") as psStp,
            tc.tile_pool(name="dram", bufs=1, space="DRAM") as dram,
        ):
            eb = EngineBalancer(nc)
            eb.act_unit = act_unit
            # ---- resident loads ------------------------------------------------
            w1sb = res.tile([128, D // 128, 3 * FPC], dt.bfloat16, tag="w1")
            nc.sync.dma_start(w1sb[:], w1l.ap().rearrange("(a p) c -> p a c", p=128))
            b1sb = res.tile([FPC, 3], dt.float32, tag="b1")
            nc.sync.dma_start(b1sb[:], b1l.ap().rearrange("m p -> p m"))
            identsb = res.tile([SB, SB], dt.bfloat16, tag="ident")
            nc.sync.dma_start(identsb[:], ident.ap())
            xsb = res.tile([128, D // 128, T], dt.bfloat16, tag="x", name="xsb")
            for kc in range(D // 128):
                nc.sync.dma_start(xsb[:, kc, :],
                                  xT[kc * 128:(kc + 1) * 128, :])
            masksb = res.tile([SB, 4, TC], dt.float32, tag="masks")
            nc.sync.dma_start(masksb[:], masks.ap().rearrange("r p c -> p r c"))
            w2sb = res.tile([128, D // 128, D], dt.bfloat16, tag="w2")
            b2row = res.tile([1, D], dt.float32, tag="b2row")
            b2bc = res.tile([128, D], dt.float32, tag="b2bc")
            nc.sync.dma_start(w2sb[:], w2.ap().rearrange("(a p) c -> p a c", p=128))
            nc.sync.dma_start(b2row[:], b2.ap().rearrange("(o c) -> o c", o=1))
            nc.gpsimd.partition_broadcast(b2bc[:], b2row[:])

            kTsb = res.tile([FPC, T], dt.bfloat16, tag="kT")
            qTsb = res.tile([FPC, T], dt.bfloat16, tag="qT")
            vsb = res.tile([SB, T // SB, FPC], dt.bfloat16, tag="v")
            # k in [t, d] layout (for the state update); last chunk not needed
            knT = res.tile([SB, (NT - 1) * 4, FPC], dt.bfloat16, tag="knT")
            # running state: both heads packed block-diagonally
            psSt = psStp.tile([128, 128], dt.float32, tag="st", name="psSt")

            # A2A exchange groups: (chunk list, shard width, out row base).
            groups = GROUPS_SPEC[groups_mode]
            a2a_in, a2a_out = [], []
            for g, (chunks, W, base) in enumerate(groups):
                a2a_in.append(dram.tile([NCORES, FPC, W], dt.bfloat16,
                                        tag=f"ain{g}", name=f"a2a_in{g}"))
                a2a_out.append(dram.tile([NCORES, FPC, W], dt.bfloat16,
                                         tag=f"aout{g}", name=f"a2a_out{g}"))
            chunk_group = {}
            for g, (chunks, W, base) in enumerate(groups):
                for pos, i in enumerate(chunks):
                    chunk_group[i] = (g, pos)

            def a2a_launch(g: int):
                if fake_cc:
                    nc.gpsimd.dma_start(a2a_out[g][:], a2a_in[g][:])
                    return
                nc.gpsimd.collective_compute(
                    "AllToAll", mybir.AluOpType.bypass, replica_groups=rg,
                    ins=[a2a_in[g].opt()], outs=[a2a_out[g].opt()],
                )

            def rx_load(g: int):
                W = groups[g][1]
                rxt = []
                for fc in range(NCORES):
                    r = rxp.tile([FPC, W], dt.bfloat16, tag="rx",
                                 name=f"rx{g}_{fc}")
                    nc.sync.dma_start(r[:], a2a_out[g][fc])
                    rxt.append(r)
                return rxt

            def att_store(i, att):
                g, pos = chunk_group[i]
                W = groups[g][1]
                a = 512 * pos
                while a < 512 * pos + TC:
                    s = a // W
                    e = min((s + 1) * W, 512 * pos + TC)
                    nc.sync.dma_start(
                        a2a_in[g][s][:, a - s * W: e - s * W],
                        att[:, a - 512 * pos: e - 512 * pos])
                    a = e

            def w2_group(g: int, rxt, pool=None, tag="acc"):
                """Project group g's rows through W2 (after its AllToAll)."""
                chunks, W, base = groups[g]
                pool = pool or psA
                for tt in range(W // 128):
                    for ec in range(2):
                        psF = pool.tile([128, 512], dt.float32, tag=tag,
                                        name=f"psF_{g}_{tt}_{ec}")
                        for fc in range(NCORES):
                            nc.tensor.matmul(
                                psF[:], rxt[fc][:, ts(tt, 128)],
                                w2sb[:, fc, ts(ec, 512)],
                                start=(fc == 0), stop=(fc == NCORES - 1),
                            )
                        o = ostp.tile([128, 512], dt.bfloat16, tag="ost",
                                      name=f"o_{g}_{tt}_{ec}")
                        eb.tensor_add(o[:], psF[:], b2bc[:, ts(ec, 512)], 512)
                        nc.sync.dma_start(
                            out[base + tt * 128: base + tt * 128 + 128,
                                ts(ec, 512)], o[:])

            def phase_a(i: int):
                """kqv^T for t-chunk i."""
                xts = [xsb[:, kc, ts(i, TC)] for kc in range(D // 128)]
                for m in range(3):
                    ps = psA.tile([128, TC], dt.float32, tag="acc",
                                  name=f"psa_{i}_{m}")
                    for kc in range(D // 128):
                        nc.tensor.matmul(
                            ps[:], w1sb[:, kc, ts(m, FPC)], xts[kc],
                            start=(kc == 0), stop=(kc == D // 128 - 1),
                        )
                    if m == 0:
                        eb.bias_add(kTsb[:, ts(i, TC)], ps[:],
                                    b1sb[:, m:m + 1], TC)
                        if i < NT - 1:
                            # k in [t, d] layout: the xbar transpose engine
                            # is serial (~1.3us/tile), so split the load
                            # between it and PE transposes staged through
                            # the psO ring (idle during phase_a)
                            for tb in range(4):
                                src = kTsb[:, i * TC + tb * SB:
                                           i * TC + (tb + 1) * SB]
                                if tb % 2 == 0:
                                    nc.sync.dma_start_transpose(
                                        knT[:, 4 * i + tb, :], src)
                                else:
                                    pstk = psO.tile(
                                        [SB, SB], dt.bfloat16, tag="ot",
                                        name=f"pstk_{i}_{tb}")
                                    nc.tensor.transpose(
                                        pstk[:], src, identsb[:])
                                    eb.copy(knT[:, 4 * i + tb, :],
                                            pstk[:], 128)
                    elif m == 1:
                        eb.bias_add(qTsb[:, ts(i, TC)], ps[:],
                                    b1sb[:, m:m + 1], TC)
                    else:
                        vtmp = stage.tile([128, TC], dt.bfloat16, tag="vtmp",
                                          name=f"vtmp_{i}")
                        eb.bias_add(vtmp[:], ps[:], b1sb[:, m:m + 1], TC)
                        for u in range(TC // SB):
                            pst = psA.tile([SB, SB], dt.bfloat16, tag="acc",
                                           name=f"pst_{i}_{u}")
                            nc.tensor.transpose(
                                pst[:], vtmp[:, ts(u, SB)], identsb[:])
                            eb.copy(vsb[:, 4 * i + u, :], pst[:], 128)

            def attention(i: int):
                otp = psO.tile([128, TC], dt.float32, tag="ot",
                               name=f"otp_{i}")
                # ---- cross-chunk context via the running state -------------
                if i > 0:
                    st = stp.tile([128, 128], dt.bfloat16, tag="st",
                                  name=f"st_{i}")
                    eb.copy(st[:], psSt[:], 128)
                    nc.tensor.matmul(
                        otp[0:64, :], st[0:64, 0:64],
                        qTsb[0:64, ts(i, TC)], start=True, stop=True)
                    nc.tensor.matmul(
                        otp[64:128, :], st[64:128, 64:128],
                        qTsb[64:128, ts(i, TC)], start=True, stop=True)
                # ---- in-chunk diagonal blocks ------------------------------
                for r in range(4):
                    j = 4 * i + r
                    cs = 128 * r
                    s0p = psS0.tile([128, TC], dt.float32, tag="s0",
                                    name=f"s0p_{i}_{r}")
                    s1p = psS1.tile([128, TC], dt.float32, tag="s1",
                                    name=f"s1p_{i}_{r}")
                    nc.tensor.matmul(
                        s0p[:, cs:TC], kTsb[0:64, ts(j, SB)],
                        qTsb[0:64, i * TC + cs:(i + 1) * TC],
                        start=True, stop=True, tile_position=(0, 0))
                    nc.tensor.matmul(
                        s1p[:, cs:TC], kTsb[64:128, ts(j, SB)],
                        qTsb[64:128, i * TC + cs:(i + 1) * TC],
                        start=True, stop=True, tile_position=(64, 0))
                    s0s = sp0.tile([128, TC], dt.bfloat16, tag="s0s",
                                   name=f"s0s_{i}_{r}")
                    s1s = sp1.tile([128, TC], dt.bfloat16, tag="s1s",
                                   name=f"s1s_{i}_{r}")
                    # the triangle lives in cols [128r, 128r+128); the rest of
                    # the row is a plain (engine-balanced) copy.
                    me = min(cs + 128, TC)
                    eb.mul_mask(s0s[:, cs:me], s0p[:, cs:me],
                                masksb[:, r, cs:me], me - cs)
                    eb.mul_mask(s1s[:, cs:me], s1p[:, cs:me],
                                masksb[:, r, cs:me], me - cs)
                    if me < TC:
                        eb.copy(s0s[:, me:TC], s0p[:, me:TC], TC - me)
                        eb.copy(s1s[:, me:TC], s1p[:, me:TC], TC - me)
                    nc.tensor.matmul(
                        otp[0:64, cs:TC], vsb[:, j, 0:64], s0s[:, cs:TC],
                        start=(i == 0 and r == 0), stop=(r == 3),
                        tile_position=(0, 0))
                    nc.tensor.matmul(
                        otp[64:128, cs:TC], vsb[:, j, 64:128], s1s[:, cs:TC],
                        start=(i == 0 and r == 0), stop=(r == 3),
                        tile_position=(0, 64))
                # ---- state update with this chunk's keys/values ------------
                if i < NT - 1:
                    for tb in range(4):
                        nc.tensor.matmul(
                            psSt[:], knT[:, 4 * i + tb, :],
                            vsb[:, 4 * i + tb, :],
                            start=(i == 0 and tb == 0),
                            stop=(i == NT - 2 and tb == 3))

                att = stage.tile([128, TC], dt.bfloat16, tag="att",
                                 name=f"att_{i}")
                eb.copy(att[:], otp[:], TC)
                att_store(i, att)

            # ---- schedule ------------------------------------------------------
            for _rep in range(reps):
                phase_a(0)
                phase_a(1)
                rxt0 = None
                for i in range(NT):
                    attention(i)
                    if i + 2 < NT:
                        phase_a(i + 2)
                    if i == 3:
                        a2a_launch(0)
                    if i == 4:
                        rxt0 = rx_load(0)
                    if i == 5:
                        w2_group(0, rxt0)
                a2a_launch(1)
                rxt1 = rx_load(1)
                # tail W2 group stages through the psO ring so the next
                # rep's projection chunks get the accumulator ring at once
                w2_group(1, rxt1, pool=psO, tag="ot")

    nc.compile()
    return nc


_GROUPS_MODE = "asc"
_PROGRAM_CACHE: list = []


def _get_program() -> bass.Bass:
    if not _PROGRAM_CACHE:
        _PROGRAM_CACHE.append(build_program(groups_mode=_GROUPS_MODE))
    return _PROGRAM_CACHE[0]


def _make_in_maps(x, W1, b1, W2, b2):
    import ml_dtypes
    bf16 = ml_dtypes.bfloat16
    xTb = np.ascontiguousarray(np.asarray(x, np.float32).T).astype(bf16)
    w2b = np.asarray(W2, np.float32).astype(bf16)
    b2f = np.ascontiguousarray(b2, dtype=np.float32)
    maskt = np.zeros((4, SB, TC), dtype=np.float32)
    for r in range(4):
        for p in range(SB):
            maskt[r, p, 128 * r + p:] = 1.0
    identm = np.eye(SB, dtype=np.float32).astype(bf16)

    in_maps = []
    for c in range(NCORES):
        cols = slice(FPC * c, FPC * (c + 1))
        w1c = np.concatenate(
            [W1[:, 0 * D:][:, cols], W1[:, 1 * D:][:, cols], W1[:, 2 * D:][:, cols]],
            axis=1)
        b1c = np.stack(
            [b1[0 * D:][cols], b1[1 * D:][cols], b1[2 * D:][cols]], axis=0)
        in_maps.append({
            "xT": xTb,
            "w1l": np.asarray(w1c, np.float32).astype(bf16),
            "b1l": np.ascontiguousarray(b1c, dtype=np.float32),
            "w2": w2b,
            "b2": b2f,
            "masks": maskt,
            "ident": identm,
        })
    return in_maps


def kernel(x, W1, b1, W2, b2, _trace=False, **trace_kwargs):
    x = np.asarray(x)
    W1, b1, W2, b2 = (np.asarray(a) for a in (W1, b1, W2, b2))
    nc = _get_program()
    in_maps = _make_in_maps(x, W1, b1, W2, b2)
    res = run_bass_kernel_spmd(
        nc, in_maps, core_ids=list(range(NCORES)), trace=_trace, **trace_kwargs)
    full = np.empty((T, D), dtype=np.float32)
    groups = GROUPS_SPEC[_GROUPS_MODE]
    for c in range(NCORES):
        o = np.asarray(res.results[c]["out"], dtype=np.float32)
        for chunks, W, base in groups:
            L = np.arange(W * c, W * c + W)
            tglob = 512 * np.asarray(chunks)[L // 512] + (L % 512)
            full[tglob] = o[base: base + W]
    if _trace:
        kernel.last_results = res
    return full
